# revision 1
# baseline (speedup 1.0000x reference)
"""DynamicEdgeConv GNN (3x EdgeConv + encoder) on 8 TRN2 NeuronCores.

Data-parallel over graphs: 16 graphs of 2048 nodes; 2 graphs per core.
Per graph-conv: hT [H=128, N=2048] kept feature-major in SBUF.
  scores(i,j) = h_i . h_j - 0.5*||h_j||^2   (argtop8 == kNN by distance)
  top-8 via DVE max / max_index, gather rows via indirect DMA from a DRAM
  copy of h, message MLP via PE with the [xi, xj-xi] concat rewritten as
  xi@(A-B) + xj@B, max-aggregate over k via DVE tensor_reduce on a strided
  view. Bias of the encoder is folded as a K=5 matmul; -0.5||h_j||^2 is
  folded as a K=1 ones matmul into the scores PSUM group.
"""

import numpy as np
from contextlib import ExitStack

import concourse.bass as bass
import concourse.mybir as mybir
from concourse import tile
from concourse.masks import make_identity

B_ALL = 16      # graphs total
N = 2048        # nodes per graph
KNN = 8
H = 128
F_IN = 4
CORES = 8
GPC = B_ALL // CORES          # graphs per core
NPC = GPC * N                 # nodes per core
NCH = N // 128                # 16 chunks of 128 nodes per graph
NB = N // 512                 # 4 blocks of 512 nodes per graph

FP = mybir.dt.float32
FR = mybir.dt.float32r
F16 = mybir.dt.float16
U32 = mybir.dt.uint32


def fp(ap):
    return ap.bitcast(FP)


# weights consumed as f32r matmul operands (DMA'd straight into f32r tiles);
# biases consumed by ACT stay fp32
FR_WEIGHTS = {"W_enc", "b_enc", "W1a", "W1b", "W2a", "W2b", "W5a", "W5b"}
AF = mybir.ActivationFunctionType
ALU = mybir.AluOpType
AX = mybir.AxisListType

CONV_TAGS = ["1", "2", "5"]

WEIGHT_SPECS = {
    "W_enc": (F_IN, H), "b_enc": (1, H),
    "W1a": (2 * H, H), "b1a": (H, 1), "W1b": (H, H), "b1b": (H, 1),
    "W2a": (2 * H, H), "b2a": (H, 1), "W2b": (H, H), "b2b": (H, 1),
    "W5a": (2 * H, H), "b5a": (H, 1), "W5b": (H, 1), "b5b": (1, 1),
}


def emit(tc, x, out_d, W):
    nc = tc.nc
    with ExitStack() as ctx:
        consts = ctx.enter_context(tc.tile_pool(name="consts", bufs=1))
        hpool = ctx.enter_context(tc.tile_pool(name="hpool", bufs=3))
        work = ctx.enter_context(tc.tile_pool(name="work", bufs=2))
        # deep pool for the small per-k MLP tiles: the k-chain is
        # latency-bound, so 4-deep rotation lets k+2/k+3 start early
        mlpp = ctx.enter_context(tc.tile_pool(name="mlpp", bufs=4))
        strips = ctx.enter_context(tc.tile_pool(name="strips", bufs=1))
        psum = ctx.enter_context(tc.tile_pool(name="psum", bufs=2, space="PSUM"))
        hdram = ctx.enter_context(tc.tile_pool(name="hdram", bufs=1, space="DRAM"))

        ident = consts.tile([128, 128], FP, tag="ident", name="ident")
        make_identity(nc, ident)
        ones_cf = consts.tile([128, 1], FP, tag="ones_cf", name="ones_cf")
        nc.vector.memset(ones_cf, 1.0)
        ones_col = consts.tile([128, 1], FR, tag="ones_col", name="ones_col")
        nc.scalar.activation(ones_col, ones_cf, AF.Copy)
        ones_5f = consts.tile([1, 512], FP, tag="ones_5f", name="ones_5f")
        nc.vector.memset(ones_5f, 1.0)
        ones_512 = consts.tile([1, 512], FR, tag="ones_512", name="ones_512")
        nc.scalar.activation(ones_512, ones_5f, AF.Copy)
        ones_row = consts.tile([1, 128], FR, tag="ones_row", name="ones_row")
        nc.scalar.activation(ones_row, ones_5f[:, 0:128], AF.Copy)

        w_enc_sb = consts.tile([F_IN, H], FR, tag="w_enc_sb", name="w_enc_sb")
        nc.sync.dma_start(w_enc_sb, W["W_enc"])
        b_enc_sb = consts.tile([1, H], FR, tag="b_enc_sb", name="b_enc_sb")
        nc.sync.dma_start(b_enc_sb, W["b_enc"])

        convW = []
        for t in CONV_TAGS:
            AB = consts.tile([H, 2 * H], FR, tag=f"AB{t}", name=f"AB{t}")
            nc.sync.dma_start(AB.rearrange("h (a j) -> h a j", a=2),
                              W[f"W{t}a"].rearrange("(a h) j -> h a j", a=2))
            Bm = AB[:, H:2 * H]
            AmB = consts.tile([H, H], FR, tag=f"AmB{t}", name=f"AmB{t}")
            nc.vector.tensor_sub(AmB, AB[:, 0:H], Bm)
            ba = consts.tile([H, 1], FP, tag=f"ba{t}", name=f"ba{t}")
            nc.sync.dma_start(ba, W[f"b{t}a"])
            if t != "5":
                Wb = consts.tile([H, H], FR, tag=f"Wb{t}", name=f"Wb{t}")
                bb = consts.tile([H, 1], FP, tag=f"bb{t}", name=f"bb{t}")
            else:
                Wb = consts.tile([H, 1], FR, tag=f"Wb{t}", name=f"Wb{t}")
                bb = consts.tile([1, 1], FP, tag=f"bb{t}", name=f"bb{t}")
            nc.sync.dma_start(Wb, W[f"W{t}b"])
            nc.sync.dma_start(bb, W[f"b{t}b"])
            convW.append((AmB, Bm, ba, Wb, bb))

        h_tab = [[hdram.tile([N, H], FP, tag=f"ht_{g}_{c}", name=f"ht_{g}_{c}")
                  for c in range(3)] for g in range(GPC)]

        # x transposed into SBUF (feature-major)
        xT = consts.tile([F_IN, NPC], FR, tag="xT", name="xT")
        nc.sync.dma_start(xT, x.rearrange("n f -> f n"))

        def store_htab(g, c, hT_src):
            dst = h_tab[g][c].rearrange("(cb q p) f -> cb p q f", q=4, p=128)
            for cb in range(4):
                pst = psum.tile([128, 512], FP, tag="t", name="pst_st")
                for q in range(4):
                    col = (cb * 4 + q) * 128
                    nc.tensor.transpose(pst[:, q * 128:(q + 1) * 128],
                                        fp(hT_src[:, col:col + 128]), ident)
                hsb = work.tile([128, 512], FP, tag="hst", name="hsb")
                nc.scalar.activation(hsb, pst, AF.Copy)
                nc.sync.dma_start(dst[cb], hsb.rearrange("p (q f) -> p q f", q=4))

        def edge_conv(g, conv, hT_in):
            AmB, Bm, ba, Wb, bb = convW[conv]

            h2 = work.tile([H, N], FR, tag="h2", name="h2")
            nc.scalar.activation(h2, fp(hT_in), AF.Square)
            neghalf = strips.tile([1, N], FR, tag="nh", name="neghalf")
            for jb in range(NB):
                ps = psum.tile([128, 512], FP, tag="s", name="ps_sq")
                nc.tensor.matmul(ps[0:1, :], ones_col,
                                 h2[:, jb * 512:(jb + 1) * 512],
                                 start=True, stop=True)
                nc.scalar.activation(neghalf[:, jb * 512:(jb + 1) * 512], ps[0:1, :],
                                     AF.Copy, scale=-0.5)

            # unique idx tile per (g, conv): avoids WAR waits from the 8
            # SWDGE gather queues landing on max_index (1-wait-slot limit)
            idx = consts.tile([128, NCH * KNN], U32, tag=f"idx_{g}_{conv}",
                              name=f"idx_{g}_{conv}")
            def emit_scores(ib):
                for q in range(4):
                    ci = ib * 4 + q
                    # fp16 scores: halves the DVE top-8 scan cost; only the
                    # argmax selection consumes these values
                    sc = work.tile([128, N], F16, tag="sc", name="sc")
                    for jb in range(NB):
                        ps = psum.tile([128, 512], FP, tag="s", name="ps_sc")
                        nc.tensor.matmul(ps, hT_in[:, ci * 128:(ci + 1) * 128],
                                         hT_in[:, jb * 512:(jb + 1) * 512],
                                         start=True, stop=False)
                        nc.tensor.matmul(ps, ones_row,
                                         neghalf[:, jb * 512:(jb + 1) * 512],
                                         start=False, stop=True)
                        nc.scalar.activation(sc[:, jb * 512:(jb + 1) * 512], ps,
                                             AF.Copy)
                    vals = work.tile([128, 8], F16, tag="vals", name="vals")
                    nc.vector.max(vals, sc)
                    nc.vector.max_index(idx[:, ci * KNN:(ci + 1) * KNN], vals, sc)

            if conv < 2:
                hT_out = hpool.tile([H, N], FR, tag="hT", name="hT_out")
            else:
                out_row = strips.tile([1, N], FP, tag="outrow", name="out_row")
            emit_scores(0)
            for ib in range(NB):
                # software pipeline: PE computes next block's scores while this
                # block's top-8 + gathers drain on DVE/SWDGE
                if ib + 1 < NB:
                    emit_scores(ib + 1)
                if conv < 2:
                    msgs = work.tile([128, KNN * 512], FP, tag="msgs", name="msgs")
                else:
                    m5 = strips.tile([1, KNN * 512], FP, tag="m5", name="m5")
                for k in range(KNN):
                    pst = psum.tile([128, 512], FP, tag="t", name="pst_xj")
                    for q in range(4):
                        ci = ib * 4 + q
                        # per-q tags: 4 gathers per k would otherwise ping-pong
                        # on a 2-deep buffer, serializing Pool behind PE
                        xj = mlpp.tile([128, H], FP, tag=f"xj{q}", name=f"xj{q}")
                        nc.gpsimd.indirect_dma_start(
                            out=xj, out_offset=None,
                            in_=h_tab[g][conv],
                            in_offset=bass.IndirectOffsetOnAxis(
                                ap=idx[:, ci * KNN + k: ci * KNN + k + 1], axis=0),
                        )
                        nc.tensor.transpose(pst[:, q * 128:(q + 1) * 128], xj, ident)
                    xjT = mlpp.tile([H, 512], FR, tag="xjT", name="xjT")
                    nc.scalar.activation(xjT, pst, AF.Copy)
                    ps1 = psum.tile([128, 512], FP, tag="m1", name="ps1")
                    nc.tensor.matmul(ps1, Bm, xjT, start=True, stop=False)
                    nc.tensor.matmul(ps1, AmB,
                                     hT_in[:, ib * 512:(ib + 1) * 512],
                                     start=False, stop=True)
                    h1 = mlpp.tile([H, 512], FR, tag="h1", name="h1")
                    nc.scalar.activation(h1, ps1, AF.Relu, bias=ba)
                    if conv < 2:
                        ps2 = psum.tile([128, 512], FP, tag="m2", name="ps2")
                        nc.tensor.matmul(ps2, Wb, h1, start=True, stop=True)
                        nc.scalar.activation(msgs[:, k * 512:(k + 1) * 512], ps2,
                                             AF.Relu, bias=bb)
                    else:
                        ps2 = psum.tile([1, 512], FP, tag="m2", name="ps2s")
                        nc.tensor.matmul(ps2, Wb, h1, start=True, stop=True)
                        nc.scalar.activation(m5[:, k * 512:(k + 1) * 512], ps2,
                                             AF.Relu, bias=bb)
                if conv < 2:
                    nc.vector.tensor_reduce(
                        out=hT_out[:, ib * 512:(ib + 1) * 512],
                        in_=msgs.rearrange("p (k i) -> p i k", k=KNN),
                        axis=AX.X, op=ALU.max)
                else:
                    nc.vector.tensor_reduce(
                        out=out_row[:, ib * 512:(ib + 1) * 512],
                        in_=m5.rearrange("p (k i) -> p i k", k=KNN),
                        axis=AX.X, op=ALU.max)
            if conv < 2:
                store_htab(g, conv + 1, hT_out)
                return hT_out
            # sigmoid after max (monotone), then store this graph's 2048 outputs
            sg_row = strips.tile([1, N], FP, tag="sgrow", name="sg_row")
            nc.scalar.activation(sg_row, out_row, AF.Sigmoid)
            dst = out_d.rearrange("(g n) one -> g one n", g=GPC)
            nc.sync.dma_start(dst[g], sg_row)
            return None

        for g in range(GPC):
            hT_cur = hpool.tile([H, N], FR, tag="hT", name="hT_enc")
            for jb in range(NB):
                ps = psum.tile([128, 512], FP, tag="s", name="ps_enc")
                nc.tensor.matmul(ps, w_enc_sb,
                                 xT[:, g * N + jb * 512: g * N + (jb + 1) * 512],
                                 start=True, stop=False)
                nc.tensor.matmul(ps, b_enc_sb, ones_512,
                                 start=False, stop=True)
                nc.scalar.activation(hT_cur[:, jb * 512:(jb + 1) * 512], ps, AF.Copy)
            store_htab(g, 0, hT_cur)
            for conv in range(3):
                hT_cur = edge_conv(g, conv, hT_cur)


def build():
    nc = bass.Bass("TRN2", target_bir_lowering=False, debug=False)
    x_d = nc.dram_tensor("x", [NPC, F_IN], FR, kind="ExternalInput")
    w_aps = {}
    for name, shape in WEIGHT_SPECS.items():
        dt = FR if name in FR_WEIGHTS else FP
        w_aps[name] = nc.dram_tensor(name, list(shape), dt, kind="ExternalInput")[:]
    out_d = nc.dram_tensor("out", [NPC, 1], FP, kind="ExternalOutput")
    with tile.TileContext(nc) as tc:
        emit(tc, x_d[:], out_d[:], w_aps)
    # walrus CoreV3 codegen allows at most 1 sync wait per instruction;
    # split multi-wait instructions via event semaphores (Bacc passes)
    import bass_rust
    bass_rust.move_matmul_waits_to_ldweights(nc.m)
    bass_rust.generate_event_semaphores(nc)
    return nc


def make_in_maps(inputs):
    def f32(a):
        return np.ascontiguousarray(np.asarray(a), dtype=np.float32)
    w = {name: f32(inputs[name]).reshape(shape)
         for name, shape in WEIGHT_SPECS.items()}
    x_full = f32(inputs["x"])
    in_maps = []
    for c in range(CORES):
        m = dict(w)
        m["x"] = np.ascontiguousarray(x_full[c * NPC:(c + 1) * NPC])
        in_maps.append(m)
    return in_maps


def run(inputs, trace=False):
    from concourse.bass_utils import run_bass_kernel_spmd
    nc = build()
    in_maps = make_in_maps(inputs)
    res = run_bass_kernel_spmd(nc, in_maps, list(range(CORES)), trace=trace)
    out = np.concatenate(
        [np.asarray(res.results[c]["out"], dtype=np.float32) for c in range(CORES)],
        axis=0)
    return out, res


def kernel(**inputs):
    out, _ = run(inputs, trace=False)
    return out



# revision 28
# speedup vs baseline: 1.8145x; 1.8145x over previous
"""DynamicEdgeConv GNN (3x EdgeConv + encoder) on 8 TRN2 NeuronCores.

Data-parallel over graphs: 16 graphs of 2048 nodes; 2 graphs per core.
Per graph-conv: hT [H=128, N=2048] kept feature-major in SBUF.
  scores(i,j) = h_i . h_j - 0.5*||h_j||^2   (argtop8 == kNN by distance)
  top-8 via DVE max / max_index, gather rows via indirect DMA from a DRAM
  copy of h, message MLP via PE with the [xi, xj-xi] concat rewritten as
  xi@(A-B) + xj@B, max-aggregate over k via DVE tensor_reduce on a strided
  view. Bias of the encoder is folded as a K=5 matmul; -0.5||h_j||^2 is
  folded as a K=1 ones matmul into the scores PSUM group.
"""

import numpy as np
from contextlib import ExitStack

import concourse.bass as bass
import concourse.mybir as mybir
from concourse import tile
from concourse.masks import make_identity

B_ALL = 16      # graphs total
N = 2048        # nodes per graph
KNN = 8
H = 128
F_IN = 4
CORES = 8
GPC = B_ALL // CORES          # graphs per core
NPC = GPC * N                 # nodes per core
NCH = N // 128                # 16 chunks of 128 nodes per graph
NB = N // 512                 # 4 blocks of 512 nodes per graph

FP = mybir.dt.float32
FR = mybir.dt.float32r
F16 = mybir.dt.float16
U32 = mybir.dt.uint32


def fp(ap):
    return ap.bitcast(FP)


AF = mybir.ActivationFunctionType
ALU = mybir.AluOpType
AX = mybir.AxisListType

CONV_TAGS = ["1", "2", "5"]

WEIGHT_SPECS = {
    "W_enc": (F_IN, H), "b_enc": (1, H),
    "W1a": (2 * H, H), "b1a": (H, 1), "W1b": (H, H), "b1b": (H, 1),
    "W2a": (2 * H, H), "b2a": (H, 1), "W2b": (H, H), "b2b": (H, 1),
    "W5a": (2 * H, H), "b5a": (H, 1), "W5b": (H, 1), "b5b": (1, 1),
}

# Everything the device needs rides in ONE fp16 tensor per core: the
# per-call wall time here is dominated by host->device transfer over the
# axon tunnel (~13 ms/MB + ~2 ms per tensor), so 17 fp32 arrays ->
# 1 packed fp16 blob roughly halves the end-to-end latency.
PACK_ORDER = [("x", (NPC, F_IN))] + [(k, WEIGHT_SPECS[k])
                                     for k in WEIGHT_SPECS]
PACK_OFF = {}
_off = 0
for _nm, _shp in PACK_ORDER:
    PACK_OFF[_nm] = _off
    _off += int(np.prod(_shp))
BLOB_LEN = _off


def emit(tc, x, out_d, W):
    nc = tc.nc
    with ExitStack() as ctx:
        consts = ctx.enter_context(tc.tile_pool(name="consts", bufs=1))
        hpool = ctx.enter_context(tc.tile_pool(name="hpool", bufs=3))
        work = ctx.enter_context(tc.tile_pool(name="work", bufs=2))
        # deep pool for the small per-k MLP tiles: the k-chain is
        # latency-bound, so 4-deep rotation lets k+2/k+3 start early
        mlpp = ctx.enter_context(tc.tile_pool(name="mlpp", bufs=4))
        strips = ctx.enter_context(tc.tile_pool(name="strips", bufs=1))
        psum = ctx.enter_context(tc.tile_pool(name="psum", bufs=2, space="PSUM"))
        hdram = ctx.enter_context(tc.tile_pool(name="hdram", bufs=1, space="DRAM"))

        ident = consts.tile([128, 128], FP, tag="ident", name="ident")
        make_identity(nc, ident)
        ones_cf = consts.tile([128, 1], FP, tag="ones_cf", name="ones_cf")
        nc.vector.memset(ones_cf, 1.0)
        ones_col = consts.tile([128, 1], FR, tag="ones_col", name="ones_col")
        nc.scalar.activation(ones_col, ones_cf, AF.Copy)
        ones_5f = consts.tile([1, 512], FP, tag="ones_5f", name="ones_5f")
        nc.vector.memset(ones_5f, 1.0)
        ones_512 = consts.tile([1, 512], FR, tag="ones_512", name="ones_512")
        nc.scalar.activation(ones_512, ones_5f, AF.Copy)
        ones_row = consts.tile([1, 128], FR, tag="ones_row", name="ones_row")
        nc.scalar.activation(ones_row, ones_5f[:, 0:128], AF.Copy)

        # fp16 staging + upconvert of the packed weights (once per call)
        def load_fr(name, shape, out_dt=FR, view=None):
            src = W[name] if view is None else view
            t16 = consts.tile(list(shape), F16, tag=f"{name}_16",
                              name=f"{name}_16")
            nc.sync.dma_start(t16, src)
            out = consts.tile(list(shape), out_dt, tag=f"{name}_sb",
                              name=f"{name}_sb")
            nc.scalar.activation(out, t16, AF.Copy)
            return out

        w_enc_sb = load_fr("W_enc", (F_IN, H))
        b_enc_sb = load_fr("b_enc", (1, H))

        convW = []
        for t in CONV_TAGS:
            AB16 = consts.tile([H, 2 * H], F16, tag=f"AB{t}_16",
                               name=f"AB{t}_16")
            nc.sync.dma_start(AB16.rearrange("h (a j) -> h a j", a=2),
                              W[f"W{t}a"].rearrange("(a h) j -> h a j", a=2))
            AB = consts.tile([H, 2 * H], FR, tag=f"AB{t}", name=f"AB{t}")
            nc.scalar.activation(AB, AB16, AF.Copy)
            Bm = AB[:, H:2 * H]
            AmB = consts.tile([H, H], FR, tag=f"AmB{t}", name=f"AmB{t}")
            nc.vector.tensor_sub(AmB, AB[:, 0:H], Bm)
            ba = load_fr(f"b{t}a", (H, 1), out_dt=FP)
            if t != "5":
                Wb = load_fr(f"W{t}b", (H, H))
                bb = load_fr(f"b{t}b", (H, 1), out_dt=FP)
            else:
                Wb = load_fr(f"W{t}b", (H, 1))
                bb = load_fr(f"b{t}b", (1, 1), out_dt=FP)
            convW.append((AmB, Bm, ba, Wb, bb))

        h_tab = [[hdram.tile([N, H], FP, tag=f"ht_{g}_{c}", name=f"ht_{g}_{c}")
                  for c in range(3)] for g in range(GPC)]

        # x transposed into SBUF (feature-major), fp16 staged
        xT16 = consts.tile([F_IN, NPC], F16, tag="xT16", name="xT16")
        nc.sync.dma_start(xT16, x.rearrange("n f -> f n"))
        xT = consts.tile([F_IN, NPC], FR, tag="xT", name="xT")
        nc.scalar.activation(xT, xT16, AF.Copy)

        def store_htab(g, c, hT_src):
            dst = h_tab[g][c].rearrange("(cb q p) f -> cb p q f", q=4, p=128)
            for cb in range(4):
                pst = psum.tile([128, 512], FP, tag="t", name="pst_st")
                for q in range(4):
                    col = (cb * 4 + q) * 128
                    nc.tensor.transpose(pst[:, q * 128:(q + 1) * 128],
                                        fp(hT_src[:, col:col + 128]), ident)
                hsb = work.tile([128, 512], FP, tag="hst", name="hsb")
                nc.scalar.activation(hsb, pst, AF.Copy)
                nc.sync.dma_start(dst[cb], hsb.rearrange("p (q f) -> p q f", q=4))

        def edge_conv(g, conv, hT_in):
            AmB, Bm, ba, Wb, bb = convW[conv]

            h2 = work.tile([H, N], FR, tag="h2", name="h2")
            nc.scalar.activation(h2, fp(hT_in), AF.Square)
            neghalf = strips.tile([1, N], FR, tag="nh", name="neghalf")
            for jb in range(NB):
                ps = psum.tile([128, 512], FP, tag="s", name="ps_sq")
                nc.tensor.matmul(ps[0:1, :], ones_col,
                                 h2[:, jb * 512:(jb + 1) * 512],
                                 start=True, stop=True)
                nc.scalar.activation(neghalf[:, jb * 512:(jb + 1) * 512], ps[0:1, :],
                                     AF.Copy, scale=-0.5)

            # unique idx tile per (g, conv): avoids WAR waits from the 8
            # SWDGE gather queues landing on max_index (1-wait-slot limit)
            idx = consts.tile([128, NCH * KNN], U32, tag=f"idx_{g}_{conv}",
                              name=f"idx_{g}_{conv}")
            def emit_scores(ib):
                for q in range(4):
                    ci = ib * 4 + q
                    # fp16 scores: halves the DVE top-8 scan cost; only the
                    # argmax selection consumes these values
                    sc = work.tile([128, N], F16, tag="sc", name="sc")
                    for jb in range(NB):
                        ps = psum.tile([128, 512], FP, tag="s", name="ps_sc")
                        nc.tensor.matmul(ps, hT_in[:, ci * 128:(ci + 1) * 128],
                                         hT_in[:, jb * 512:(jb + 1) * 512],
                                         start=True, stop=False)
                        nc.tensor.matmul(ps, ones_row,
                                         neghalf[:, jb * 512:(jb + 1) * 512],
                                         start=False, stop=True)
                        nc.scalar.activation(sc[:, jb * 512:(jb + 1) * 512], ps,
                                             AF.Copy)
                    vals = work.tile([128, 8], F16, tag="vals", name="vals")
                    nc.vector.max(vals, sc)
                    nc.vector.max_index(idx[:, ci * KNN:(ci + 1) * KNN], vals, sc)

            if conv < 2:
                hT_out = hpool.tile([H, N], FR, tag="hT", name="hT_out")
            else:
                out_row = strips.tile([1, N], FP, tag="outrow", name="out_row")
            emit_scores(0)
            for ib in range(NB):
                # software pipeline: PE computes next block's scores while this
                # block's top-8 + gathers drain on DVE/SWDGE
                if ib + 1 < NB:
                    emit_scores(ib + 1)
                if conv < 2:
                    msgs = work.tile([128, KNN * 512], FP, tag="msgs", name="msgs")
                else:
                    m5 = strips.tile([1, KNN * 512], FP, tag="m5", name="m5")
                for k in range(KNN):
                    pst = psum.tile([128, 512], FP, tag="t", name="pst_xj")
                    for q in range(4):
                        ci = ib * 4 + q
                        # per-q tags: 4 gathers per k would otherwise ping-pong
                        # on a 2-deep buffer, serializing Pool behind PE
                        xj = mlpp.tile([128, H], FP, tag=f"xj{q}", name=f"xj{q}")
                        nc.gpsimd.indirect_dma_start(
                            out=xj, out_offset=None,
                            in_=h_tab[g][conv],
                            in_offset=bass.IndirectOffsetOnAxis(
                                ap=idx[:, ci * KNN + k: ci * KNN + k + 1], axis=0),
                        )
                        nc.tensor.transpose(pst[:, q * 128:(q + 1) * 128], xj, ident)
                    xjT = mlpp.tile([H, 512], FR, tag="xjT", name="xjT")
                    nc.scalar.activation(xjT, pst, AF.Copy)
                    ps1 = psum.tile([128, 512], FP, tag="m1", name="ps1")
                    nc.tensor.matmul(ps1, Bm, xjT, start=True, stop=False)
                    nc.tensor.matmul(ps1, AmB,
                                     hT_in[:, ib * 512:(ib + 1) * 512],
                                     start=False, stop=True)
                    h1 = mlpp.tile([H, 512], FR, tag="h1", name="h1")
                    nc.scalar.activation(h1, ps1, AF.Relu, bias=ba)
                    if conv < 2:
                        ps2 = psum.tile([128, 512], FP, tag="m2", name="ps2")
                        nc.tensor.matmul(ps2, Wb, h1, start=True, stop=True)
                        nc.scalar.activation(msgs[:, k * 512:(k + 1) * 512], ps2,
                                             AF.Relu, bias=bb)
                    else:
                        ps2 = psum.tile([1, 512], FP, tag="m2", name="ps2s")
                        nc.tensor.matmul(ps2, Wb, h1, start=True, stop=True)
                        nc.scalar.activation(m5[:, k * 512:(k + 1) * 512], ps2,
                                             AF.Relu, bias=bb)
                if conv < 2:
                    nc.vector.tensor_reduce(
                        out=hT_out[:, ib * 512:(ib + 1) * 512],
                        in_=msgs.rearrange("p (k i) -> p i k", k=KNN),
                        axis=AX.X, op=ALU.max)
                else:
                    nc.vector.tensor_reduce(
                        out=out_row[:, ib * 512:(ib + 1) * 512],
                        in_=m5.rearrange("p (k i) -> p i k", k=KNN),
                        axis=AX.X, op=ALU.max)
            if conv < 2:
                store_htab(g, conv + 1, hT_out)
                return hT_out
            # sigmoid after max (monotone), then store this graph's 2048 outputs
            sg_row = strips.tile([1, N], FP, tag="sgrow", name="sg_row")
            nc.scalar.activation(sg_row, out_row, AF.Sigmoid)
            dst = out_d.rearrange("(g n) one -> g one n", g=GPC)
            nc.sync.dma_start(dst[g], sg_row)
            return None

        for g in range(GPC):
            hT_cur = hpool.tile([H, N], FR, tag="hT", name="hT_enc")
            for jb in range(NB):
                ps = psum.tile([128, 512], FP, tag="s", name="ps_enc")
                nc.tensor.matmul(ps, w_enc_sb,
                                 xT[:, g * N + jb * 512: g * N + (jb + 1) * 512],
                                 start=True, stop=False)
                nc.tensor.matmul(ps, b_enc_sb, ones_512,
                                 start=False, stop=True)
                nc.scalar.activation(hT_cur[:, jb * 512:(jb + 1) * 512], ps, AF.Copy)
            store_htab(g, 0, hT_cur)
            for conv in range(3):
                hT_cur = edge_conv(g, conv, hT_cur)


def build():
    nc = bass.Bass("TRN2", target_bir_lowering=False, debug=False)
    blob_d = nc.dram_tensor("blob", [BLOB_LEN], F16, kind="ExternalInput")
    views = {}
    for name, shape in PACK_ORDER:
        off = PACK_OFF[name]
        n = int(np.prod(shape))
        views[name] = blob_d[off:off + n].rearrange(
            "(a b) -> a b", a=shape[0], b=shape[1])
    out_d = nc.dram_tensor("out", [NPC, 1], FP, kind="ExternalOutput")
    with tile.TileContext(nc) as tc:
        emit(tc, views["x"], out_d[:], views)
    # walrus CoreV3 codegen allows at most 1 sync wait per instruction;
    # split multi-wait instructions via event semaphores (Bacc passes)
    import bass_rust
    bass_rust.move_matmul_waits_to_ldweights(nc.m)
    bass_rust.generate_event_semaphores(nc)
    return nc


def make_in_maps(inputs):
    wblob = np.empty(BLOB_LEN, np.float16)
    for name, shape in PACK_ORDER:
        if name == "x":
            continue
        off = PACK_OFF[name]
        n = int(np.prod(shape))
        wblob[off:off + n] = np.asarray(inputs[name], dtype=np.float16).reshape(-1)
    x_full = np.asarray(inputs["x"], dtype=np.float16).reshape(NPC * CORES, F_IN)
    in_maps = []
    nx = NPC * F_IN
    for c in range(CORES):
        b = wblob.copy()
        b[0:nx] = x_full[c * NPC:(c + 1) * NPC].reshape(-1)
        in_maps.append({"blob": b})
    return in_maps


def run(inputs, trace=False):
    from concourse.bass_utils import run_bass_kernel_spmd
    nc = build()
    in_maps = make_in_maps(inputs)
    res = run_bass_kernel_spmd(nc, in_maps, list(range(CORES)), trace=trace)
    out = np.concatenate(
        [np.asarray(res.results[c]["out"], dtype=np.float32) for c in range(CORES)],
        axis=0)
    return out, res


def kernel(**inputs):
    out, _ = run(inputs, trace=False)
    return out



# revision 29
# speedup vs baseline: 1.8203x; 1.0032x over previous
"""DynamicEdgeConv GNN (3x EdgeConv + encoder) on 8 TRN2 NeuronCores.

Data-parallel over graphs: 16 graphs of 2048 nodes; 2 graphs per core.
Per graph-conv: hT [H=128, N=2048] kept feature-major in SBUF.
  scores(i,j) = h_i . h_j - 0.5*||h_j||^2   (argtop8 == kNN by distance)
  top-8 via DVE max / max_index, gather rows via indirect DMA from a DRAM
  copy of h, message MLP via PE with the [xi, xj-xi] concat rewritten as
  xi@(A-B) + xj@B, max-aggregate over k via DVE tensor_reduce on a strided
  view. Bias of the encoder is folded as a K=5 matmul; -0.5||h_j||^2 is
  folded as a K=1 ones matmul into the scores PSUM group.

End-to-end latency here is transfer-bound, not compute-bound (device exec
is ~0.7 ms/core; each invocation pays a fixed dispatch cost plus
~13 ms/MB of host->device input traffic, with ~2 ms per separate input
tensor). All inputs therefore ship as ONE packed fp16 blob per core
(x shard + weights, upconverted to f32r on device), which halves the
steady-state wall time vs 17 separate fp32 arrays.
"""

import numpy as np
from contextlib import ExitStack

import concourse.bass as bass
import concourse.mybir as mybir
from concourse import tile
from concourse.masks import make_identity

B_ALL = 16      # graphs total
N = 2048        # nodes per graph
KNN = 8
H = 128
F_IN = 4
CORES = 8
GPC = B_ALL // CORES          # graphs per core
NPC = GPC * N                 # nodes per core
NCH = N // 128                # 16 chunks of 128 nodes per graph
NB = N // 512                 # 4 blocks of 512 nodes per graph

FP = mybir.dt.float32
FR = mybir.dt.float32r
F16 = mybir.dt.float16
U32 = mybir.dt.uint32


def fp(ap):
    return ap.bitcast(FP)


AF = mybir.ActivationFunctionType
ALU = mybir.AluOpType
AX = mybir.AxisListType

CONV_TAGS = ["1", "2", "5"]

WEIGHT_SPECS = {
    "W_enc": (F_IN, H), "b_enc": (1, H),
    "W1a": (2 * H, H), "b1a": (H, 1), "W1b": (H, H), "b1b": (H, 1),
    "W2a": (2 * H, H), "b2a": (H, 1), "W2b": (H, H), "b2b": (H, 1),
    "W5a": (2 * H, H), "b5a": (H, 1), "W5b": (H, 1), "b5b": (1, 1),
}

# Everything the device needs rides in ONE fp16 tensor per core: the
# per-call wall time here is dominated by host->device transfer over the
# axon tunnel (~13 ms/MB + ~2 ms per tensor), so 17 fp32 arrays ->
# 1 packed fp16 blob roughly halves the end-to-end latency.
PACK_ORDER = [("x", (NPC, F_IN))] + [(k, WEIGHT_SPECS[k])
                                     for k in WEIGHT_SPECS]
PACK_OFF = {}
_off = 0
for _nm, _shp in PACK_ORDER:
    PACK_OFF[_nm] = _off
    _off += int(np.prod(_shp))
BLOB_LEN = _off


def emit(tc, x, out_d, W):
    nc = tc.nc
    with ExitStack() as ctx:
        consts = ctx.enter_context(tc.tile_pool(name="consts", bufs=1))
        hpool = ctx.enter_context(tc.tile_pool(name="hpool", bufs=3))
        work = ctx.enter_context(tc.tile_pool(name="work", bufs=2))
        # deep pool for the small per-k MLP tiles: the k-chain is
        # latency-bound, so 4-deep rotation lets k+2/k+3 start early
        mlpp = ctx.enter_context(tc.tile_pool(name="mlpp", bufs=4))
        strips = ctx.enter_context(tc.tile_pool(name="strips", bufs=1))
        psum = ctx.enter_context(tc.tile_pool(name="psum", bufs=2, space="PSUM"))
        hdram = ctx.enter_context(tc.tile_pool(name="hdram", bufs=1, space="DRAM"))

        ident = consts.tile([128, 128], FP, tag="ident", name="ident")
        make_identity(nc, ident)
        ones_cf = consts.tile([128, 1], FP, tag="ones_cf", name="ones_cf")
        nc.vector.memset(ones_cf, 1.0)
        ones_col = consts.tile([128, 1], FR, tag="ones_col", name="ones_col")
        nc.scalar.activation(ones_col, ones_cf, AF.Copy)
        ones_5f = consts.tile([1, 512], FP, tag="ones_5f", name="ones_5f")
        nc.vector.memset(ones_5f, 1.0)
        ones_512 = consts.tile([1, 512], FR, tag="ones_512", name="ones_512")
        nc.scalar.activation(ones_512, ones_5f, AF.Copy)
        ones_row = consts.tile([1, 128], FR, tag="ones_row", name="ones_row")
        nc.scalar.activation(ones_row, ones_5f[:, 0:128], AF.Copy)

        # fp16 staging + upconvert of the packed weights (once per call)
        def load_fr(name, shape, out_dt=FR, view=None):
            src = W[name] if view is None else view
            t16 = consts.tile(list(shape), F16, tag=f"{name}_16",
                              name=f"{name}_16")
            nc.sync.dma_start(t16, src)
            out = consts.tile(list(shape), out_dt, tag=f"{name}_sb",
                              name=f"{name}_sb")
            nc.scalar.activation(out, t16, AF.Copy)
            return out

        w_enc_sb = load_fr("W_enc", (F_IN, H))
        b_enc_sb = load_fr("b_enc", (1, H))

        convW = []
        for t in CONV_TAGS:
            AB16 = consts.tile([H, 2 * H], F16, tag=f"AB{t}_16",
                               name=f"AB{t}_16")
            nc.sync.dma_start(AB16.rearrange("h (a j) -> h a j", a=2),
                              W[f"W{t}a"].rearrange("(a h) j -> h a j", a=2))
            AB = consts.tile([H, 2 * H], FR, tag=f"AB{t}", name=f"AB{t}")
            nc.scalar.activation(AB, AB16, AF.Copy)
            Bm = AB[:, H:2 * H]
            AmB = consts.tile([H, H], FR, tag=f"AmB{t}", name=f"AmB{t}")
            nc.vector.tensor_sub(AmB, AB[:, 0:H], Bm)
            ba = load_fr(f"b{t}a", (H, 1), out_dt=FP)
            if t != "5":
                Wb = load_fr(f"W{t}b", (H, H))
                bb = load_fr(f"b{t}b", (H, 1), out_dt=FP)
            else:
                Wb = load_fr(f"W{t}b", (H, 1))
                bb = load_fr(f"b{t}b", (1, 1), out_dt=FP)
            convW.append((AmB, Bm, ba, Wb, bb))

        h_tab = [[hdram.tile([N, H], FP, tag=f"ht_{g}_{c}", name=f"ht_{g}_{c}")
                  for c in range(3)] for g in range(GPC)]

        # x transposed into SBUF (feature-major), fp16 staged
        xT16 = consts.tile([F_IN, NPC], F16, tag="xT16", name="xT16")
        nc.sync.dma_start(xT16, x.rearrange("n f -> f n"))
        xT = consts.tile([F_IN, NPC], FR, tag="xT", name="xT")
        nc.scalar.activation(xT, xT16, AF.Copy)

        def store_htab(g, c, hT_src):
            dst = h_tab[g][c].rearrange("(cb q p) f -> cb p q f", q=4, p=128)
            for cb in range(4):
                pst = psum.tile([128, 512], FP, tag="t", name="pst_st")
                for q in range(4):
                    col = (cb * 4 + q) * 128
                    nc.tensor.transpose(pst[:, q * 128:(q + 1) * 128],
                                        fp(hT_src[:, col:col + 128]), ident)
                hsb = work.tile([128, 512], FP, tag="hst", name="hsb")
                nc.scalar.activation(hsb, pst, AF.Copy)
                nc.sync.dma_start(dst[cb], hsb.rearrange("p (q f) -> p q f", q=4))

        def edge_conv(g, conv, hT_in):
            AmB, Bm, ba, Wb, bb = convW[conv]

            h2 = work.tile([H, N], FR, tag="h2", name="h2")
            nc.scalar.activation(h2, fp(hT_in), AF.Square)
            neghalf = strips.tile([1, N], FR, tag="nh", name="neghalf")
            for jb in range(NB):
                ps = psum.tile([128, 512], FP, tag="s", name="ps_sq")
                nc.tensor.matmul(ps[0:1, :], ones_col,
                                 h2[:, jb * 512:(jb + 1) * 512],
                                 start=True, stop=True)
                nc.scalar.activation(neghalf[:, jb * 512:(jb + 1) * 512], ps[0:1, :],
                                     AF.Copy, scale=-0.5)

            # unique idx tile per (g, conv): avoids WAR waits from the 8
            # SWDGE gather queues landing on max_index (1-wait-slot limit)
            idx = consts.tile([128, NCH * KNN], U32, tag=f"idx_{g}_{conv}",
                              name=f"idx_{g}_{conv}")
            def emit_scores(ib):
                for q in range(4):
                    ci = ib * 4 + q
                    # fp16 scores: halves the DVE top-8 scan cost; only the
                    # argmax selection consumes these values
                    sc = work.tile([128, N], F16, tag="sc", name="sc")
                    for jb in range(NB):
                        ps = psum.tile([128, 512], FP, tag="s", name="ps_sc")
                        nc.tensor.matmul(ps, hT_in[:, ci * 128:(ci + 1) * 128],
                                         hT_in[:, jb * 512:(jb + 1) * 512],
                                         start=True, stop=False)
                        nc.tensor.matmul(ps, ones_row,
                                         neghalf[:, jb * 512:(jb + 1) * 512],
                                         start=False, stop=True)
                        nc.scalar.activation(sc[:, jb * 512:(jb + 1) * 512], ps,
                                             AF.Copy)
                    vals = work.tile([128, 8], F16, tag="vals", name="vals")
                    nc.vector.max(vals, sc)
                    nc.vector.max_index(idx[:, ci * KNN:(ci + 1) * KNN], vals, sc)

            if conv < 2:
                hT_out = hpool.tile([H, N], FR, tag="hT", name="hT_out")
            else:
                out_row = strips.tile([1, N], FP, tag="outrow", name="out_row")
            emit_scores(0)
            for ib in range(NB):
                # software pipeline: PE computes next block's scores while this
                # block's top-8 + gathers drain on DVE/SWDGE
                if ib + 1 < NB:
                    emit_scores(ib + 1)
                if conv < 2:
                    msgs = work.tile([128, KNN * 512], FP, tag="msgs", name="msgs")
                else:
                    m5 = strips.tile([1, KNN * 512], FP, tag="m5", name="m5")
                for k in range(KNN):
                    pst = psum.tile([128, 512], FP, tag="t", name="pst_xj")
                    for q in range(4):
                        ci = ib * 4 + q
                        # per-q tags: 4 gathers per k would otherwise ping-pong
                        # on a 2-deep buffer, serializing Pool behind PE
                        xj = mlpp.tile([128, H], FP, tag=f"xj{q}", name=f"xj{q}")
                        nc.gpsimd.indirect_dma_start(
                            out=xj, out_offset=None,
                            in_=h_tab[g][conv],
                            in_offset=bass.IndirectOffsetOnAxis(
                                ap=idx[:, ci * KNN + k: ci * KNN + k + 1], axis=0),
                        )
                        nc.tensor.transpose(pst[:, q * 128:(q + 1) * 128], xj, ident)
                    xjT = mlpp.tile([H, 512], FR, tag="xjT", name="xjT")
                    nc.scalar.activation(xjT, pst, AF.Copy)
                    ps1 = psum.tile([128, 512], FP, tag="m1", name="ps1")
                    nc.tensor.matmul(ps1, Bm, xjT, start=True, stop=False)
                    nc.tensor.matmul(ps1, AmB,
                                     hT_in[:, ib * 512:(ib + 1) * 512],
                                     start=False, stop=True)
                    h1 = mlpp.tile([H, 512], FR, tag="h1", name="h1")
                    nc.scalar.activation(h1, ps1, AF.Relu, bias=ba)
                    if conv < 2:
                        ps2 = psum.tile([128, 512], FP, tag="m2", name="ps2")
                        nc.tensor.matmul(ps2, Wb, h1, start=True, stop=True)
                        nc.scalar.activation(msgs[:, k * 512:(k + 1) * 512], ps2,
                                             AF.Relu, bias=bb)
                    else:
                        ps2 = psum.tile([1, 512], FP, tag="m2", name="ps2s")
                        nc.tensor.matmul(ps2, Wb, h1, start=True, stop=True)
                        nc.scalar.activation(m5[:, k * 512:(k + 1) * 512], ps2,
                                             AF.Relu, bias=bb)
                if conv < 2:
                    nc.vector.tensor_reduce(
                        out=hT_out[:, ib * 512:(ib + 1) * 512],
                        in_=msgs.rearrange("p (k i) -> p i k", k=KNN),
                        axis=AX.X, op=ALU.max)
                else:
                    nc.vector.tensor_reduce(
                        out=out_row[:, ib * 512:(ib + 1) * 512],
                        in_=m5.rearrange("p (k i) -> p i k", k=KNN),
                        axis=AX.X, op=ALU.max)
            if conv < 2:
                store_htab(g, conv + 1, hT_out)
                return hT_out
            # sigmoid after max (monotone), then store this graph's 2048 outputs
            sg_row = strips.tile([1, N], FP, tag="sgrow", name="sg_row")
            nc.scalar.activation(sg_row, out_row, AF.Sigmoid)
            dst = out_d.rearrange("(g n) one -> g one n", g=GPC)
            nc.sync.dma_start(dst[g], sg_row)
            return None

        for g in range(GPC):
            hT_cur = hpool.tile([H, N], FR, tag="hT", name="hT_enc")
            for jb in range(NB):
                ps = psum.tile([128, 512], FP, tag="s", name="ps_enc")
                nc.tensor.matmul(ps, w_enc_sb,
                                 xT[:, g * N + jb * 512: g * N + (jb + 1) * 512],
                                 start=True, stop=False)
                nc.tensor.matmul(ps, b_enc_sb, ones_512,
                                 start=False, stop=True)
                nc.scalar.activation(hT_cur[:, jb * 512:(jb + 1) * 512], ps, AF.Copy)
            store_htab(g, 0, hT_cur)
            for conv in range(3):
                hT_cur = edge_conv(g, conv, hT_cur)


def build():
    nc = bass.Bass("TRN2", target_bir_lowering=False, debug=False)
    blob_d = nc.dram_tensor("blob", [BLOB_LEN], F16, kind="ExternalInput")
    views = {}
    for name, shape in PACK_ORDER:
        off = PACK_OFF[name]
        n = int(np.prod(shape))
        views[name] = blob_d[off:off + n].rearrange(
            "(a b) -> a b", a=shape[0], b=shape[1])
    out_d = nc.dram_tensor("out", [NPC, 1], FP, kind="ExternalOutput")
    with tile.TileContext(nc) as tc:
        emit(tc, views["x"], out_d[:], views)
    # walrus CoreV3 codegen allows at most 1 sync wait per instruction;
    # split multi-wait instructions via event semaphores (Bacc passes)
    import bass_rust
    bass_rust.move_matmul_waits_to_ldweights(nc.m)
    bass_rust.generate_event_semaphores(nc)
    return nc


def make_in_maps(inputs):
    wblob = np.empty(BLOB_LEN, np.float16)
    for name, shape in PACK_ORDER:
        if name == "x":
            continue
        off = PACK_OFF[name]
        n = int(np.prod(shape))
        wblob[off:off + n] = np.asarray(inputs[name], dtype=np.float16).reshape(-1)
    x_full = np.asarray(inputs["x"], dtype=np.float16).reshape(NPC * CORES, F_IN)
    in_maps = []
    nx = NPC * F_IN
    for c in range(CORES):
        b = wblob.copy()
        b[0:nx] = x_full[c * NPC:(c + 1) * NPC].reshape(-1)
        in_maps.append({"blob": b})
    return in_maps


def run(inputs, trace=False):
    from concourse.bass_utils import run_bass_kernel_spmd
    nc = build()
    in_maps = make_in_maps(inputs)
    res = run_bass_kernel_spmd(nc, in_maps, list(range(CORES)), trace=trace)
    out = np.concatenate(
        [np.asarray(res.results[c]["out"], dtype=np.float32) for c in range(CORES)],
        axis=0)
    return out, res


def kernel(**inputs):
    out, _ = run(inputs, trace=False)
    return out



# revision 33
# speedup vs baseline: 2.1709x; 1.1926x over previous
"""DynamicEdgeConv GNN (3x EdgeConv + encoder) on 8 TRN2 NeuronCores.

Data-parallel over graphs: 16 graphs of 2048 nodes; 2 graphs per core.
Per graph-conv: hT [H=128, N=2048] kept feature-major in SBUF.
  scores(i,j) = h_i . h_j - 0.5*||h_j||^2   (argtop8 == kNN by distance)
  top-8 via DVE max / max_index, gather rows via indirect DMA from a DRAM
  copy of h, message MLP via PE with the [xi, xj-xi] concat rewritten as
  xi@(A-B) + xj@B, max-aggregate over k via DVE tensor_reduce on a strided
  view. Bias of the encoder is folded as a K=5 matmul; -0.5||h_j||^2 is
  folded as a K=1 ones matmul into the scores PSUM group.

End-to-end latency here is transfer-bound, not compute-bound (device exec
is ~0.7 ms/core; each invocation pays a fixed dispatch cost plus
~13 ms/MB of host->device input traffic, with ~2 ms per separate input
tensor). All inputs therefore ship as ONE packed fp16 blob per core
(x shard + weights, upconverted to f32r on device), which halves the
steady-state wall time vs 17 separate fp32 arrays.
"""

import numpy as np
from contextlib import ExitStack

import concourse.bass as bass
import concourse.mybir as mybir
from concourse import tile
from concourse.masks import make_identity

B_ALL = 16      # graphs total
N = 2048        # nodes per graph
KNN = 8
H = 128
F_IN = 4
# 1 core: per-invocation wall cost is dominated by mesh dispatch + input
# transfer, both of which shrink with fewer cores (no 8x weight
# replication, no 8-way sync over the tunnel); 16 graphs of device work
# is still only ~6 ms
CORES = 1
GPC = B_ALL // CORES          # graphs per core
NPC = GPC * N                 # nodes per core
NCH = N // 128                # 16 chunks of 128 nodes per graph
NB = N // 512                 # 4 blocks of 512 nodes per graph

FP = mybir.dt.float32
FR = mybir.dt.float32r
F16 = mybir.dt.float16
U32 = mybir.dt.uint32


def fp(ap):
    return ap.bitcast(FP)


AF = mybir.ActivationFunctionType
ALU = mybir.AluOpType
AX = mybir.AxisListType

CONV_TAGS = ["1", "2", "5"]

WEIGHT_SPECS = {
    "W_enc": (F_IN, H), "b_enc": (1, H),
    "W1a": (2 * H, H), "b1a": (H, 1), "W1b": (H, H), "b1b": (H, 1),
    "W2a": (2 * H, H), "b2a": (H, 1), "W2b": (H, H), "b2b": (H, 1),
    "W5a": (2 * H, H), "b5a": (H, 1), "W5b": (H, 1), "b5b": (1, 1),
}

# Everything the device needs rides in ONE fp16 tensor per core: the
# per-call wall time here is dominated by host->device transfer over the
# axon tunnel (~13 ms/MB + ~2 ms per tensor), so 17 fp32 arrays ->
# 1 packed fp16 blob roughly halves the end-to-end latency.
PACK_ORDER = [("x", (NPC, F_IN))] + [(k, WEIGHT_SPECS[k])
                                     for k in WEIGHT_SPECS]
PACK_OFF = {}
_off = 0
for _nm, _shp in PACK_ORDER:
    PACK_OFF[_nm] = _off
    _off += int(np.prod(_shp))
BLOB_LEN = _off


def emit(tc, x, out_d, W):
    nc = tc.nc
    with ExitStack() as ctx:
        consts = ctx.enter_context(tc.tile_pool(name="consts", bufs=1))
        hpool = ctx.enter_context(tc.tile_pool(name="hpool", bufs=3))
        work = ctx.enter_context(tc.tile_pool(name="work", bufs=2))
        # deep pool for the small per-k MLP tiles: the k-chain is
        # latency-bound, so 4-deep rotation lets k+2/k+3 start early
        mlpp = ctx.enter_context(tc.tile_pool(name="mlpp", bufs=4))
        strips = ctx.enter_context(tc.tile_pool(name="strips", bufs=1))
        psum = ctx.enter_context(tc.tile_pool(name="psum", bufs=2, space="PSUM"))
        hdram = ctx.enter_context(tc.tile_pool(name="hdram", bufs=1, space="DRAM"))

        ident = consts.tile([128, 128], FP, tag="ident", name="ident")
        make_identity(nc, ident)
        ones_cf = consts.tile([128, 1], FP, tag="ones_cf", name="ones_cf")
        nc.vector.memset(ones_cf, 1.0)
        ones_col = consts.tile([128, 1], FR, tag="ones_col", name="ones_col")
        nc.scalar.activation(ones_col, ones_cf, AF.Copy)
        ones_5f = consts.tile([1, 512], FP, tag="ones_5f", name="ones_5f")
        nc.vector.memset(ones_5f, 1.0)
        ones_512 = consts.tile([1, 512], FR, tag="ones_512", name="ones_512")
        nc.scalar.activation(ones_512, ones_5f, AF.Copy)
        ones_row = consts.tile([1, 128], FR, tag="ones_row", name="ones_row")
        nc.scalar.activation(ones_row, ones_5f[:, 0:128], AF.Copy)

        # fp16 staging + upconvert of the packed weights (once per call)
        def load_fr(name, shape, out_dt=FR, view=None):
            src = W[name] if view is None else view
            t16 = consts.tile(list(shape), F16, tag=f"{name}_16",
                              name=f"{name}_16")
            nc.sync.dma_start(t16, src)
            out = consts.tile(list(shape), out_dt, tag=f"{name}_sb",
                              name=f"{name}_sb")
            nc.scalar.activation(out, t16, AF.Copy)
            return out

        w_enc_sb = load_fr("W_enc", (F_IN, H))
        b_enc_sb = load_fr("b_enc", (1, H))

        convW = []
        for t in CONV_TAGS:
            AB16 = consts.tile([H, 2 * H], F16, tag=f"AB{t}_16",
                               name=f"AB{t}_16")
            nc.sync.dma_start(AB16.rearrange("h (a j) -> h a j", a=2),
                              W[f"W{t}a"].rearrange("(a h) j -> h a j", a=2))
            AB = consts.tile([H, 2 * H], FR, tag=f"AB{t}", name=f"AB{t}")
            nc.scalar.activation(AB, AB16, AF.Copy)
            Bm = AB[:, H:2 * H]
            AmB = consts.tile([H, H], FR, tag=f"AmB{t}", name=f"AmB{t}")
            nc.vector.tensor_sub(AmB, AB[:, 0:H], Bm)
            ba = load_fr(f"b{t}a", (H, 1), out_dt=FP)
            if t != "5":
                Wb = load_fr(f"W{t}b", (H, H))
                bb = load_fr(f"b{t}b", (H, 1), out_dt=FP)
            else:
                Wb = load_fr(f"W{t}b", (H, 1))
                bb = load_fr(f"b{t}b", (1, 1), out_dt=FP)
            convW.append((AmB, Bm, ba, Wb, bb))

        h_tab = [[hdram.tile([N, H], FP, tag=f"ht_{g}_{c}", name=f"ht_{g}_{c}")
                  for c in range(3)] for g in range(GPC)]

        # x transposed (feature-major), fp16 staged per graph — the full
        # [4, NPC] would not fit one partition's SBUF budget at GPC=16
        xT_dram = x.rearrange("n f -> f n")

        def load_xT(g):
            xT16 = work.tile([F_IN, N], F16, tag="xT16", name="xT16")
            nc.sync.dma_start(xT16, xT_dram[:, g * N:(g + 1) * N])
            xT = work.tile([F_IN, N], FR, tag="xT", name="xT")
            nc.scalar.activation(xT, xT16, AF.Copy)
            return xT

        def store_htab(g, c, hT_src):
            dst = h_tab[g][c].rearrange("(cb q p) f -> cb p q f", q=4, p=128)
            for cb in range(4):
                pst = psum.tile([128, 512], FP, tag="t", name="pst_st")
                for q in range(4):
                    col = (cb * 4 + q) * 128
                    nc.tensor.transpose(pst[:, q * 128:(q + 1) * 128],
                                        fp(hT_src[:, col:col + 128]), ident)
                hsb = work.tile([128, 512], FP, tag="hst", name="hsb")
                nc.scalar.activation(hsb, pst, AF.Copy)
                nc.sync.dma_start(dst[cb], hsb.rearrange("p (q f) -> p q f", q=4))

        def edge_conv(g, conv, hT_in):
            AmB, Bm, ba, Wb, bb = convW[conv]

            h2 = work.tile([H, N], FR, tag="h2", name="h2")
            nc.scalar.activation(h2, fp(hT_in), AF.Square)
            neghalf = strips.tile([1, N], FR, tag="nh", name="neghalf")
            for jb in range(NB):
                ps = psum.tile([128, 512], FP, tag="s", name="ps_sq")
                nc.tensor.matmul(ps[0:1, :], ones_col,
                                 h2[:, jb * 512:(jb + 1) * 512],
                                 start=True, stop=True)
                nc.scalar.activation(neghalf[:, jb * 512:(jb + 1) * 512], ps[0:1, :],
                                     AF.Copy, scale=-0.5)

            # unique idx tile per (g, conv): avoids WAR waits from the 8
            # SWDGE gather queues landing on max_index (1-wait-slot limit)
            # unique per (g parity, conv): enough WAR distance (a full
            # graph of work) without 16 graphs x 3 convs of dead tiles
            idx = consts.tile([128, NCH * KNN], U32, tag=f"idx_{g % 2}_{conv}",
                              name=f"idx_{g % 2}_{conv}")
            def emit_scores(ib):
                for q in range(4):
                    ci = ib * 4 + q
                    # fp16 scores: halves the DVE top-8 scan cost; only the
                    # argmax selection consumes these values
                    sc = work.tile([128, N], F16, tag="sc", name="sc")
                    for jb in range(NB):
                        ps = psum.tile([128, 512], FP, tag="s", name="ps_sc")
                        nc.tensor.matmul(ps, hT_in[:, ci * 128:(ci + 1) * 128],
                                         hT_in[:, jb * 512:(jb + 1) * 512],
                                         start=True, stop=False)
                        nc.tensor.matmul(ps, ones_row,
                                         neghalf[:, jb * 512:(jb + 1) * 512],
                                         start=False, stop=True)
                        nc.scalar.activation(sc[:, jb * 512:(jb + 1) * 512], ps,
                                             AF.Copy)
                    vals = work.tile([128, 8], F16, tag="vals", name="vals")
                    nc.vector.max(vals, sc)
                    nc.vector.max_index(idx[:, ci * KNN:(ci + 1) * KNN], vals, sc)

            if conv < 2:
                hT_out = hpool.tile([H, N], FR, tag="hT", name="hT_out")
            else:
                out_row = strips.tile([1, N], FP, tag="outrow", name="out_row")
            emit_scores(0)
            for ib in range(NB):
                # software pipeline: PE computes next block's scores while this
                # block's top-8 + gathers drain on DVE/SWDGE
                if ib + 1 < NB:
                    emit_scores(ib + 1)
                if conv < 2:
                    msgs = work.tile([128, KNN * 512], FP, tag="msgs", name="msgs")
                else:
                    m5 = strips.tile([1, KNN * 512], FP, tag="m5", name="m5")
                for k in range(KNN):
                    pst = psum.tile([128, 512], FP, tag="t", name="pst_xj")
                    for q in range(4):
                        ci = ib * 4 + q
                        # per-q tags: 4 gathers per k would otherwise ping-pong
                        # on a 2-deep buffer, serializing Pool behind PE
                        xj = mlpp.tile([128, H], FP, tag=f"xj{q}", name=f"xj{q}")
                        nc.gpsimd.indirect_dma_start(
                            out=xj, out_offset=None,
                            in_=h_tab[g][conv],
                            in_offset=bass.IndirectOffsetOnAxis(
                                ap=idx[:, ci * KNN + k: ci * KNN + k + 1], axis=0),
                        )
                        nc.tensor.transpose(pst[:, q * 128:(q + 1) * 128], xj, ident)
                    xjT = mlpp.tile([H, 512], FR, tag="xjT", name="xjT")
                    nc.scalar.activation(xjT, pst, AF.Copy)
                    ps1 = psum.tile([128, 512], FP, tag="m1", name="ps1")
                    nc.tensor.matmul(ps1, Bm, xjT, start=True, stop=False)
                    nc.tensor.matmul(ps1, AmB,
                                     hT_in[:, ib * 512:(ib + 1) * 512],
                                     start=False, stop=True)
                    h1 = mlpp.tile([H, 512], FR, tag="h1", name="h1")
                    nc.scalar.activation(h1, ps1, AF.Relu, bias=ba)
                    if conv < 2:
                        ps2 = psum.tile([128, 512], FP, tag="m2", name="ps2")
                        nc.tensor.matmul(ps2, Wb, h1, start=True, stop=True)
                        nc.scalar.activation(msgs[:, k * 512:(k + 1) * 512], ps2,
                                             AF.Relu, bias=bb)
                    else:
                        ps2 = psum.tile([1, 512], FP, tag="m2", name="ps2s")
                        nc.tensor.matmul(ps2, Wb, h1, start=True, stop=True)
                        nc.scalar.activation(m5[:, k * 512:(k + 1) * 512], ps2,
                                             AF.Relu, bias=bb)
                if conv < 2:
                    nc.vector.tensor_reduce(
                        out=hT_out[:, ib * 512:(ib + 1) * 512],
                        in_=msgs.rearrange("p (k i) -> p i k", k=KNN),
                        axis=AX.X, op=ALU.max)
                else:
                    nc.vector.tensor_reduce(
                        out=out_row[:, ib * 512:(ib + 1) * 512],
                        in_=m5.rearrange("p (k i) -> p i k", k=KNN),
                        axis=AX.X, op=ALU.max)
            if conv < 2:
                store_htab(g, conv + 1, hT_out)
                return hT_out
            # sigmoid after max (monotone), then store this graph's 2048 outputs
            sg_row = strips.tile([1, N], FP, tag="sgrow", name="sg_row")
            nc.scalar.activation(sg_row, out_row, AF.Sigmoid)
            dst = out_d.rearrange("(g n) one -> g one n", g=GPC)
            nc.sync.dma_start(dst[g], sg_row)
            return None

        for g in range(GPC):
            xT = load_xT(g)
            hT_cur = hpool.tile([H, N], FR, tag="hT", name="hT_enc")
            for jb in range(NB):
                ps = psum.tile([128, 512], FP, tag="s", name="ps_enc")
                nc.tensor.matmul(ps, w_enc_sb,
                                 xT[:, jb * 512:(jb + 1) * 512],
                                 start=True, stop=False)
                nc.tensor.matmul(ps, b_enc_sb, ones_512,
                                 start=False, stop=True)
                nc.scalar.activation(hT_cur[:, jb * 512:(jb + 1) * 512], ps, AF.Copy)
            store_htab(g, 0, hT_cur)
            for conv in range(3):
                hT_cur = edge_conv(g, conv, hT_cur)


def build():
    nc = bass.Bass("TRN2", target_bir_lowering=False, debug=False)
    blob_d = nc.dram_tensor("blob", [BLOB_LEN], F16, kind="ExternalInput")
    views = {}
    for name, shape in PACK_ORDER:
        off = PACK_OFF[name]
        n = int(np.prod(shape))
        views[name] = blob_d[off:off + n].rearrange(
            "(a b) -> a b", a=shape[0], b=shape[1])
    out_d = nc.dram_tensor("out", [NPC, 1], FP, kind="ExternalOutput")
    with tile.TileContext(nc) as tc:
        emit(tc, views["x"], out_d[:], views)
    # walrus CoreV3 codegen allows at most 1 sync wait per instruction;
    # split multi-wait instructions via event semaphores (Bacc passes)
    import bass_rust
    bass_rust.move_matmul_waits_to_ldweights(nc.m)
    bass_rust.generate_event_semaphores(nc)
    return nc


def make_in_maps(inputs):
    wblob = np.empty(BLOB_LEN, np.float16)
    for name, shape in PACK_ORDER:
        if name == "x":
            continue
        off = PACK_OFF[name]
        n = int(np.prod(shape))
        wblob[off:off + n] = np.asarray(inputs[name], dtype=np.float16).reshape(-1)
    x_full = np.asarray(inputs["x"], dtype=np.float16).reshape(NPC * CORES, F_IN)
    in_maps = []
    nx = NPC * F_IN
    for c in range(CORES):
        b = wblob.copy()
        b[0:nx] = x_full[c * NPC:(c + 1) * NPC].reshape(-1)
        in_maps.append({"blob": b})
    return in_maps


def run(inputs, trace=False):
    from concourse.bass_utils import run_bass_kernel_spmd
    nc = build()
    in_maps = make_in_maps(inputs)
    res = run_bass_kernel_spmd(nc, in_maps, list(range(CORES)), trace=trace)
    out = np.concatenate(
        [np.asarray(res.results[c]["out"], dtype=np.float32) for c in range(CORES)],
        axis=0)
    return out, res


def kernel(**inputs):
    out, _ = run(inputs, trace=False)
    return out



# revision 34
# speedup vs baseline: 2.6711x; 1.2304x over previous
"""DynamicEdgeConv GNN (3x EdgeConv + encoder) on 8 TRN2 NeuronCores.

Data-parallel over graphs: 16 graphs of 2048 nodes; 2 graphs per core.
Per graph-conv: hT [H=128, N=2048] kept feature-major in SBUF.
  scores(i,j) = h_i . h_j - 0.5*||h_j||^2   (argtop8 == kNN by distance)
  top-8 via DVE max / max_index, gather rows via indirect DMA from a DRAM
  copy of h, message MLP via PE with the [xi, xj-xi] concat rewritten as
  xi@(A-B) + xj@B, max-aggregate over k via DVE tensor_reduce on a strided
  view. Bias of the encoder is folded as a K=5 matmul; -0.5||h_j||^2 is
  folded as a K=1 ones matmul into the scores PSUM group.

End-to-end latency here is transfer-bound, not compute-bound (device exec
is ~0.7 ms/core; each invocation pays a fixed dispatch cost plus
~13 ms/MB of host->device input traffic, with ~2 ms per separate input
tensor). All inputs therefore ship as ONE packed fp16 blob per core
(x shard + weights, upconverted to f32r on device), which halves the
steady-state wall time vs 17 separate fp32 arrays.
"""

import numpy as np
from contextlib import ExitStack

import concourse.bass as bass
import concourse.mybir as mybir
from concourse import tile
from concourse.masks import make_identity

B_ALL = 16      # graphs total
N = 2048        # nodes per graph
KNN = 8
H = 128
F_IN = 4
# small mesh: per-invocation wall cost is dominated by mesh dispatch +
# input transfer, both of which shrink with fewer cores (no 8x weight
# replication, no 8-way sync over the tunnel); 2 cores halves the ~6 ms
# of device work a single core would carry for one extra weight copy
CORES = 2
GPC = B_ALL // CORES          # graphs per core
NPC = GPC * N                 # nodes per core
NCH = N // 128                # 16 chunks of 128 nodes per graph
NB = N // 512                 # 4 blocks of 512 nodes per graph

FP = mybir.dt.float32
FR = mybir.dt.float32r
F16 = mybir.dt.float16
U32 = mybir.dt.uint32


def fp(ap):
    return ap.bitcast(FP)


AF = mybir.ActivationFunctionType
ALU = mybir.AluOpType
AX = mybir.AxisListType

CONV_TAGS = ["1", "2", "5"]

WEIGHT_SPECS = {
    "W_enc": (F_IN, H), "b_enc": (1, H),
    "W1a": (2 * H, H), "b1a": (H, 1), "W1b": (H, H), "b1b": (H, 1),
    "W2a": (2 * H, H), "b2a": (H, 1), "W2b": (H, H), "b2b": (H, 1),
    "W5a": (2 * H, H), "b5a": (H, 1), "W5b": (H, 1), "b5b": (1, 1),
}

# Everything the device needs rides in ONE fp16 tensor per core: the
# per-call wall time here is dominated by host->device transfer over the
# axon tunnel (~13 ms/MB + ~2 ms per tensor), so 17 fp32 arrays ->
# 1 packed fp16 blob roughly halves the end-to-end latency.
PACK_ORDER = [("x", (NPC, F_IN))] + [(k, WEIGHT_SPECS[k])
                                     for k in WEIGHT_SPECS]
PACK_OFF = {}
_off = 0
for _nm, _shp in PACK_ORDER:
    PACK_OFF[_nm] = _off
    _off += int(np.prod(_shp))
BLOB_LEN = _off


def emit(tc, x, out_d, W):
    nc = tc.nc
    with ExitStack() as ctx:
        consts = ctx.enter_context(tc.tile_pool(name="consts", bufs=1))
        hpool = ctx.enter_context(tc.tile_pool(name="hpool", bufs=3))
        work = ctx.enter_context(tc.tile_pool(name="work", bufs=2))
        # deep pool for the small per-k MLP tiles: the k-chain is
        # latency-bound, so 4-deep rotation lets k+2/k+3 start early
        mlpp = ctx.enter_context(tc.tile_pool(name="mlpp", bufs=4))
        strips = ctx.enter_context(tc.tile_pool(name="strips", bufs=1))
        psum = ctx.enter_context(tc.tile_pool(name="psum", bufs=2, space="PSUM"))
        hdram = ctx.enter_context(tc.tile_pool(name="hdram", bufs=1, space="DRAM"))

        ident = consts.tile([128, 128], FP, tag="ident", name="ident")
        make_identity(nc, ident)
        ones_cf = consts.tile([128, 1], FP, tag="ones_cf", name="ones_cf")
        nc.vector.memset(ones_cf, 1.0)
        ones_col = consts.tile([128, 1], FR, tag="ones_col", name="ones_col")
        nc.scalar.activation(ones_col, ones_cf, AF.Copy)
        ones_5f = consts.tile([1, 512], FP, tag="ones_5f", name="ones_5f")
        nc.vector.memset(ones_5f, 1.0)
        ones_512 = consts.tile([1, 512], FR, tag="ones_512", name="ones_512")
        nc.scalar.activation(ones_512, ones_5f, AF.Copy)
        ones_row = consts.tile([1, 128], FR, tag="ones_row", name="ones_row")
        nc.scalar.activation(ones_row, ones_5f[:, 0:128], AF.Copy)

        # fp16 staging + upconvert of the packed weights (once per call)
        def load_fr(name, shape, out_dt=FR, view=None):
            src = W[name] if view is None else view
            t16 = consts.tile(list(shape), F16, tag=f"{name}_16",
                              name=f"{name}_16")
            nc.sync.dma_start(t16, src)
            out = consts.tile(list(shape), out_dt, tag=f"{name}_sb",
                              name=f"{name}_sb")
            nc.scalar.activation(out, t16, AF.Copy)
            return out

        w_enc_sb = load_fr("W_enc", (F_IN, H))
        b_enc_sb = load_fr("b_enc", (1, H))

        convW = []
        for t in CONV_TAGS:
            AB16 = consts.tile([H, 2 * H], F16, tag=f"AB{t}_16",
                               name=f"AB{t}_16")
            nc.sync.dma_start(AB16.rearrange("h (a j) -> h a j", a=2),
                              W[f"W{t}a"].rearrange("(a h) j -> h a j", a=2))
            AB = consts.tile([H, 2 * H], FR, tag=f"AB{t}", name=f"AB{t}")
            nc.scalar.activation(AB, AB16, AF.Copy)
            Bm = AB[:, H:2 * H]
            AmB = consts.tile([H, H], FR, tag=f"AmB{t}", name=f"AmB{t}")
            nc.vector.tensor_sub(AmB, AB[:, 0:H], Bm)
            ba = load_fr(f"b{t}a", (H, 1), out_dt=FP)
            if t != "5":
                Wb = load_fr(f"W{t}b", (H, H))
                bb = load_fr(f"b{t}b", (H, 1), out_dt=FP)
            else:
                Wb = load_fr(f"W{t}b", (H, 1))
                bb = load_fr(f"b{t}b", (1, 1), out_dt=FP)
            convW.append((AmB, Bm, ba, Wb, bb))

        h_tab = [[hdram.tile([N, H], FP, tag=f"ht_{g}_{c}", name=f"ht_{g}_{c}")
                  for c in range(3)] for g in range(GPC)]

        # x transposed (feature-major), fp16 staged per graph — the full
        # [4, NPC] would not fit one partition's SBUF budget at GPC=16
        xT_dram = x.rearrange("n f -> f n")

        def load_xT(g):
            xT16 = work.tile([F_IN, N], F16, tag="xT16", name="xT16")
            nc.sync.dma_start(xT16, xT_dram[:, g * N:(g + 1) * N])
            xT = work.tile([F_IN, N], FR, tag="xT", name="xT")
            nc.scalar.activation(xT, xT16, AF.Copy)
            return xT

        def store_htab(g, c, hT_src):
            dst = h_tab[g][c].rearrange("(cb q p) f -> cb p q f", q=4, p=128)
            for cb in range(4):
                pst = psum.tile([128, 512], FP, tag="t", name="pst_st")
                for q in range(4):
                    col = (cb * 4 + q) * 128
                    nc.tensor.transpose(pst[:, q * 128:(q + 1) * 128],
                                        fp(hT_src[:, col:col + 128]), ident)
                hsb = work.tile([128, 512], FP, tag="hst", name="hsb")
                nc.scalar.activation(hsb, pst, AF.Copy)
                nc.sync.dma_start(dst[cb], hsb.rearrange("p (q f) -> p q f", q=4))

        def edge_conv(g, conv, hT_in):
            AmB, Bm, ba, Wb, bb = convW[conv]

            h2 = work.tile([H, N], FR, tag="h2", name="h2")
            nc.scalar.activation(h2, fp(hT_in), AF.Square)
            neghalf = strips.tile([1, N], FR, tag="nh", name="neghalf")
            for jb in range(NB):
                ps = psum.tile([128, 512], FP, tag="s", name="ps_sq")
                nc.tensor.matmul(ps[0:1, :], ones_col,
                                 h2[:, jb * 512:(jb + 1) * 512],
                                 start=True, stop=True)
                nc.scalar.activation(neghalf[:, jb * 512:(jb + 1) * 512], ps[0:1, :],
                                     AF.Copy, scale=-0.5)

            # unique idx tile per (g, conv): avoids WAR waits from the 8
            # SWDGE gather queues landing on max_index (1-wait-slot limit)
            # unique per (g parity, conv): enough WAR distance (a full
            # graph of work) without 16 graphs x 3 convs of dead tiles
            idx = consts.tile([128, NCH * KNN], U32, tag=f"idx_{g % 2}_{conv}",
                              name=f"idx_{g % 2}_{conv}")
            def emit_scores(ib):
                for q in range(4):
                    ci = ib * 4 + q
                    # fp16 scores: halves the DVE top-8 scan cost; only the
                    # argmax selection consumes these values
                    sc = work.tile([128, N], F16, tag="sc", name="sc")
                    for jb in range(NB):
                        ps = psum.tile([128, 512], FP, tag="s", name="ps_sc")
                        nc.tensor.matmul(ps, hT_in[:, ci * 128:(ci + 1) * 128],
                                         hT_in[:, jb * 512:(jb + 1) * 512],
                                         start=True, stop=False)
                        nc.tensor.matmul(ps, ones_row,
                                         neghalf[:, jb * 512:(jb + 1) * 512],
                                         start=False, stop=True)
                        nc.scalar.activation(sc[:, jb * 512:(jb + 1) * 512], ps,
                                             AF.Copy)
                    vals = work.tile([128, 8], F16, tag="vals", name="vals")
                    nc.vector.max(vals, sc)
                    nc.vector.max_index(idx[:, ci * KNN:(ci + 1) * KNN], vals, sc)

            if conv < 2:
                hT_out = hpool.tile([H, N], FR, tag="hT", name="hT_out")
            else:
                out_row = strips.tile([1, N], FP, tag="outrow", name="out_row")
            emit_scores(0)
            for ib in range(NB):
                # software pipeline: PE computes next block's scores while this
                # block's top-8 + gathers drain on DVE/SWDGE
                if ib + 1 < NB:
                    emit_scores(ib + 1)
                if conv < 2:
                    msgs = work.tile([128, KNN * 512], FP, tag="msgs", name="msgs")
                else:
                    m5 = strips.tile([1, KNN * 512], FP, tag="m5", name="m5")
                for k in range(KNN):
                    pst = psum.tile([128, 512], FP, tag="t", name="pst_xj")
                    for q in range(4):
                        ci = ib * 4 + q
                        # per-q tags: 4 gathers per k would otherwise ping-pong
                        # on a 2-deep buffer, serializing Pool behind PE
                        xj = mlpp.tile([128, H], FP, tag=f"xj{q}", name=f"xj{q}")
                        nc.gpsimd.indirect_dma_start(
                            out=xj, out_offset=None,
                            in_=h_tab[g][conv],
                            in_offset=bass.IndirectOffsetOnAxis(
                                ap=idx[:, ci * KNN + k: ci * KNN + k + 1], axis=0),
                        )
                        nc.tensor.transpose(pst[:, q * 128:(q + 1) * 128], xj, ident)
                    xjT = mlpp.tile([H, 512], FR, tag="xjT", name="xjT")
                    nc.scalar.activation(xjT, pst, AF.Copy)
                    ps1 = psum.tile([128, 512], FP, tag="m1", name="ps1")
                    nc.tensor.matmul(ps1, Bm, xjT, start=True, stop=False)
                    nc.tensor.matmul(ps1, AmB,
                                     hT_in[:, ib * 512:(ib + 1) * 512],
                                     start=False, stop=True)
                    h1 = mlpp.tile([H, 512], FR, tag="h1", name="h1")
                    nc.scalar.activation(h1, ps1, AF.Relu, bias=ba)
                    if conv < 2:
                        ps2 = psum.tile([128, 512], FP, tag="m2", name="ps2")
                        nc.tensor.matmul(ps2, Wb, h1, start=True, stop=True)
                        nc.scalar.activation(msgs[:, k * 512:(k + 1) * 512], ps2,
                                             AF.Relu, bias=bb)
                    else:
                        ps2 = psum.tile([1, 512], FP, tag="m2", name="ps2s")
                        nc.tensor.matmul(ps2, Wb, h1, start=True, stop=True)
                        nc.scalar.activation(m5[:, k * 512:(k + 1) * 512], ps2,
                                             AF.Relu, bias=bb)
                if conv < 2:
                    nc.vector.tensor_reduce(
                        out=hT_out[:, ib * 512:(ib + 1) * 512],
                        in_=msgs.rearrange("p (k i) -> p i k", k=KNN),
                        axis=AX.X, op=ALU.max)
                else:
                    nc.vector.tensor_reduce(
                        out=out_row[:, ib * 512:(ib + 1) * 512],
                        in_=m5.rearrange("p (k i) -> p i k", k=KNN),
                        axis=AX.X, op=ALU.max)
            if conv < 2:
                store_htab(g, conv + 1, hT_out)
                return hT_out
            # sigmoid after max (monotone), then store this graph's 2048 outputs
            sg_row = strips.tile([1, N], FP, tag="sgrow", name="sg_row")
            nc.scalar.activation(sg_row, out_row, AF.Sigmoid)
            dst = out_d.rearrange("(g n) one -> g one n", g=GPC)
            nc.sync.dma_start(dst[g], sg_row)
            return None

        for g in range(GPC):
            xT = load_xT(g)
            hT_cur = hpool.tile([H, N], FR, tag="hT", name="hT_enc")
            for jb in range(NB):
                ps = psum.tile([128, 512], FP, tag="s", name="ps_enc")
                nc.tensor.matmul(ps, w_enc_sb,
                                 xT[:, jb * 512:(jb + 1) * 512],
                                 start=True, stop=False)
                nc.tensor.matmul(ps, b_enc_sb, ones_512,
                                 start=False, stop=True)
                nc.scalar.activation(hT_cur[:, jb * 512:(jb + 1) * 512], ps, AF.Copy)
            store_htab(g, 0, hT_cur)
            for conv in range(3):
                hT_cur = edge_conv(g, conv, hT_cur)


def build():
    nc = bass.Bass("TRN2", target_bir_lowering=False, debug=False)
    blob_d = nc.dram_tensor("blob", [BLOB_LEN], F16, kind="ExternalInput")
    views = {}
    for name, shape in PACK_ORDER:
        off = PACK_OFF[name]
        n = int(np.prod(shape))
        views[name] = blob_d[off:off + n].rearrange(
            "(a b) -> a b", a=shape[0], b=shape[1])
    out_d = nc.dram_tensor("out", [NPC, 1], FP, kind="ExternalOutput")
    with tile.TileContext(nc) as tc:
        emit(tc, views["x"], out_d[:], views)
    # walrus CoreV3 codegen allows at most 1 sync wait per instruction;
    # split multi-wait instructions via event semaphores (Bacc passes)
    import bass_rust
    bass_rust.move_matmul_waits_to_ldweights(nc.m)
    bass_rust.generate_event_semaphores(nc)
    return nc


def make_in_maps(inputs):
    wblob = np.empty(BLOB_LEN, np.float16)
    for name, shape in PACK_ORDER:
        if name == "x":
            continue
        off = PACK_OFF[name]
        n = int(np.prod(shape))
        wblob[off:off + n] = np.asarray(inputs[name], dtype=np.float16).reshape(-1)
    x_full = np.asarray(inputs["x"], dtype=np.float16).reshape(NPC * CORES, F_IN)
    in_maps = []
    nx = NPC * F_IN
    for c in range(CORES):
        b = wblob.copy()
        b[0:nx] = x_full[c * NPC:(c + 1) * NPC].reshape(-1)
        in_maps.append({"blob": b})
    return in_maps


def run(inputs, trace=False):
    from concourse.bass_utils import run_bass_kernel_spmd
    nc = build()
    in_maps = make_in_maps(inputs)
    res = run_bass_kernel_spmd(nc, in_maps, list(range(CORES)), trace=trace)
    out = np.concatenate(
        [np.asarray(res.results[c]["out"], dtype=np.float32) for c in range(CORES)],
        axis=0)
    return out, res


def kernel(**inputs):
    out, _ = run(inputs, trace=False)
    return out

